# revision 29
# baseline (speedup 1.0000x reference)
"""MultiHeadAttention Trainium2 Bass kernel (8-core SPMD), v4.

Problem: B=2, S=2048, DIM=1024, H=16 heads (dh=64), fp32 reference.
Sharding: core c handles batch b = c//4 and 4 heads ho = 4*(c%4)..+4
(data-parallel over batch x tensor-parallel over heads). Each core:
  qhT/khT = W{q,k}.T-slice @ x.T + b   -> [256, 2048] bf16 (head-dim major)
  vh_aug  = x @ Wv.T-slice + bv (+ones col per head) per k-tile
  scores^T = kh @ qh.T (per head, K=64 row-packed pairs)
  P^T = keepmask * exp(SCALE * scores^T)                (ACT + DVE)
  out^T[65|128, q] = [vh | ones].T @ P^T                (PV + row-sums fused)
  normalize by 1/sums, y[q, :] = sum_p OT[p]-chunk.T @ Wo.T-slice

The attention inner loop is ACT-bound (exp ~2.03us/kt); everything else
is arranged to hide under it:
  - input DMAs are issue-rate-bound (~0.65us/dma_start per sequencer),
    so the head preloads are spread across the scalar/vector/gpsimd
    sequencers; sync keeps the in-loop mask stream.
  - PV matmuls are software-pipelined one kt behind the score matmuls so
    the PE FIFO never stalls waiting for exp+mask of the current kt.
  - only V proj, K chunk 0 and Q chunk 0 gate attention start; K chunks
    1-3 / Q chunks 1-3 are injected into the attention loop.
  - per-qt epilogue: po is drained to SBUF by GPSIMD (off the DVE queue)
    so the psum ring frees in ~2us; the bf16 reciprocal-broadcast
    round-trip and OT normalize run hidden under the next qt.
  - output projection at the end uses OT 128-col chunks as lhsT -> y is
    produced q-major (host gather is a plain reshape+add), alternating
    between both psum rings and both cast engines.
"""

import os
import sys

sys.path.insert(0, "/opt/trn_rl_repo")
os.environ.setdefault("MYCRO_LOCAL_CACHE", "1")

import numpy as np

import concourse.bass as bass
import concourse.bacc as bacc
import concourse.tile as tile
from concourse import mybir
from concourse import bass_utils

F32 = mybir.dt.float32
BF16 = mybir.dt.bfloat16
F8 = mybir.dt.float8e4
NP_BF16 = mybir.dt.np(BF16)
NP_F8 = mybir.dt.np(F8)
W8SCALE = 16.0  # wq/wk prescale so fp8-e4m3 stays in normal range

B, S, DIM = 2, 2048, 1024
H = 16
DH = 64
SCALE = 1.0 / (DIM ** 0.5)
N_CORES = 8
HPC = 4          # heads per core
QT = S // 512    # 4 q-chunks of 512
KT = S // 128    # 16 k-tiles of 128
CT = DIM // 128  # 8 contraction tiles for projections

# vh_aug per-kt layout: per pair p (2 local pairs):
#   A block: [vh_A(64) | ones(1)]                 at cols p*193 + [0, 65)
#   B block: [zeros(32) | ones(1) | zeros(31) | vh_B(64)] at cols p*193 + [65, 193)
#   (B ones at col +97 so B sums land on psum partition 32 -- DVE start
#   partitions must be in {0, 32, 64, 96})
VHA_W = 386


def build_nc():
    # Bacc (not plain Bass): its compile() pipeline splits multi-semaphore
    # waits into event-semaphore chains -- walrus codegen allows only ONE
    # sync wait per compute instruction on TRN2.
    nc = bacc.Bacc("TRN2", target_bir_lowering=False)

    xq_d = nc.declare_dram_parameter("xq", [QT, CT, 128, 512], F8, isOutput=False)
    xk_d = nc.declare_dram_parameter("xk", [QT, CT, 128, 512], F8, isOutput=False)
    xv_d = nc.declare_dram_parameter("xv", [QT, CT, 128, 512], BF16, isOutput=False)
    wq_d = nc.declare_dram_parameter("wq", [CT, 128, 256], F8, isOutput=False)
    wk_d = nc.declare_dram_parameter("wk", [CT, 128, 256], F8, isOutput=False)
    wv_d = nc.declare_dram_parameter("wv", [CT, 128, 256], BF16, isOutput=False)
    wo_d = nc.declare_dram_parameter("wo", [2, 128, 1024], BF16, isOutput=False)
    bq_d = nc.declare_dram_parameter("bq2", [2, 128, 1], F32, isOutput=False)
    bk_d = nc.declare_dram_parameter("bk2", [2, 128, 1], F32, isOutput=False)
    bvb_d = nc.declare_dram_parameter("bvb", [128, 256], BF16, isOutput=False)
    mk_d = nc.declare_dram_parameter("mk", [KT, QT, 128, 512], F8, isOutput=False)
    # y chunks, q-major: [qt, qc, 128 q, 1024 dims]
    yt_d = nc.declare_dram_parameter("yt", [QT, 4, 128, 1024], BF16, isOutput=True)
    rscr_d = nc.dram_tensor("rscr", [HPC, S], BF16)

    with tile.TileContext(nc) as tc:
        with tc.tile_pool(name="persist", bufs=1) as singles, \
             tc.tile_pool(name="scp", bufs=2, space="PSUM") as scp, \
             tc.tile_pool(name="pvp", bufs=2, space="PSUM") as pvp:
            def load_rows(dram, n_tiles, width, tag, eng, dt=BF16):
                tiles = []
                for c in range(n_tiles):
                    t = singles.tile([128, width], dt, tag=f"{tag}{c}", name=f"{tag}{c}")
                    eng.dma_start(out=t, in_=dram[c])
                    tiles.append(t)
                return tiles

            def alloc_x(tag, dt=BF16):
                return [[singles.tile([128, 512], dt,
                                      tag=f"{tag}{c}_{n}", name=f"{tag}{c}_{n}")
                         for n in range(QT)] for c in range(CT)]

            def load_x_chunk(dram, tiles, n, engines):
                # all 8 c-tiles of one 512-column chunk, split across engines
                for c in range(CT):
                    engines[c % len(engines)].dma_start(
                        out=tiles[c][n], in_=dram[n, c])

            xv_sb = alloc_x("xv")
            xk_sb = alloc_x("xk", dt=F8)
            xq_sb = alloc_x("xq", dt=F8)
            # DMA issue is ~0.65us/dma_start per sequencer, so spread the
            # head across sync+scalar+gpsimd. Small-but-gating tiles
            # (biases) go absolutely first; then xv (V proj), then the
            # K0/Q0 inputs; trailing weights last.
            bvb_sb = singles.tile([128, 256], BF16, tag="bvb")
            nc.sync.dma_start(out=bvb_sb, in_=bvb_d[:, :])
            bq_sb, bk_sb = [], []
            for m in range(2):
                tq = singles.tile([128, 1], F32, tag=f"bq{m}", name=f"bq{m}")
                nc.sync.dma_start(out=tq, in_=bq_d[m])
                bq_sb.append(tq)
                tk = singles.tile([128, 1], F32, tag=f"bk{m}", name=f"bk{m}")
                nc.sync.dma_start(out=tk, in_=bk_d[m])
                bk_sb.append(tk)
            wv_sb = load_rows(wv_d, CT, 256, "wv", nc.scalar)
            for n in range(QT):
                load_x_chunk(xv_d, xv_sb, n, [nc.scalar, nc.sync])
            wk_sb = load_rows(wk_d, CT, 256, "wk", nc.gpsimd, dt=F8)
            load_x_chunk(xk_d, xk_sb, 0, [nc.gpsimd])
            wq_sb = load_rows(wq_d, CT, 256, "wq", nc.gpsimd, dt=F8)
            load_x_chunk(xq_d, xq_sb, 0, [nc.gpsimd])
            load_x_chunk(xk_d, xk_sb, 1, [nc.gpsimd])
            wo_sb = load_rows(wo_d, 2, DIM, "wo", nc.sync)

            # ---- persistent intermediates ----
            qhT = [[singles.tile([128, 512], BF16, tag=f"qhT{m}_{n}",
                                 name=f"qhT{m}_{n}") for n in range(QT)]
                   for m in range(2)]
            khT = [[singles.tile([128, 512], BF16, tag=f"khT{m}_{n}",
                                 name=f"khT{m}_{n}") for n in range(QT)]
                   for m in range(2)]
            OT = [singles.tile([128, S], BF16, tag=f"OT{m}", name=f"OT{m}") for m in range(2)]
            vha = [singles.tile([128, VHA_W], BF16, tag=f"vha{kt}",
                                name=f"vha{kt}") for kt in range(KT)]

            # memsets on DVE (gpsimd's queue is busy issuing DMAs)
            warm = singles.tile([128, 1], BF16, tag="warm")
            nc.vector.memset(warm[:, :], 0.0)
            # preload the exp table set off the critical path
            dummy = singles.tile([128, 1], BF16, tag="dummy")
            nc.scalar.activation(
                out=dummy, in_=warm[:, 0:1],
                func=mybir.ActivationFunctionType.Exp, scale=1.0)
            for kt in range(KT):
                for p in range(2):
                    base = p * 193
                    nc.vector.memset(vha[kt][:, base + 64:base + 65], 1.0)
                    nc.vector.memset(vha[kt][:, base + 97:base + 98], 1.0)
                    nc.vector.memset(vha[kt][:, base + 65:base + 97], 0.0)
                    nc.vector.memset(vha[kt][:, base + 98:base + 129], 0.0)

            # ---- V projection (vh_aug tiles) ----
            def v_proj(kt, pool=None, tag="po"):
                pool = pvp if pool is None else pool
                ps = pool.tile([128, 256], F32, tag=tag, name="psv")
                for c in range(CT):
                    nc.tensor.matmul(
                        ps,
                        xv_sb[c][kt // 4][:, (kt % 4) * 128:(kt % 4 + 1) * 128],
                        wv_sb[c],
                        start=(c == 0),
                        stop=(c == CT - 1),
                    )
                for h in range(HPC):
                    p, is_b = h // 2, h % 2
                    col = p * 193 + (129 if is_b else 0)
                    nc.vector.tensor_tensor(
                        out=vha[kt][:, col:col + 64],
                        in0=ps[:, h * 64:(h + 1) * 64],
                        in1=bvb_sb[:, h * 64:(h + 1) * 64],
                        op=mybir.AluOpType.add,
                    )

            def proj_qk(w_sb, b_sb, dst, m, n, x_sb):
                ps = scp.tile([128, 512], F32, tag="sc", name="psqk")
                for c in range(CT):
                    nc.tensor.matmul(
                        ps,
                        w_sb[c][:, m * 128:(m + 1) * 128],
                        x_sb[c][n],
                        start=(c == 0),
                        stop=(c == CT - 1),
                    )
                bb = b_sb[m][:, 0:1]
                bb_bc = bass.AP(
                    tensor=bb.tensor, offset=bb.offset,
                    ap=[list(bb.ap[0]), [0, 512]])
                nc.vector.tensor_tensor(
                    out=dst[m][n],
                    in0=ps,
                    in1=bb_bc,
                    op=mybir.AluOpType.add,
                )

            # ---- head: V interleaved with K chunk 0 + Q chunk 0; the rest
            # of K/Q is injected into the attention loop ----
            for kt in range(8):
                v_proj(kt)
            proj_qk(wk_sb, bk_sb, khT, 0, 0, xk_sb)
            for kt in range(8, 12):
                v_proj(kt)
            proj_qk(wk_sb, bk_sb, khT, 1, 0, xk_sb)
            for kt in range(12, KT):
                v_proj(kt)
            for m in range(2):
                proj_qk(wq_sb, bq_sb, qhT, m, 0, xq_sb)

            def pv_step(po, pt_pair, kt):
                for p in range(2):
                    base = p * 193
                    nc.tensor.matmul(
                        po[p][:, 0:512],
                        vha[kt][:, base:base + 128],
                        pt_pair[p][:, 0:512],
                        start=(kt == 0), stop=(kt == KT - 1),
                    )
                    nc.tensor.matmul(
                        po[p][:, 512:1024],
                        vha[kt][:, base + 65:base + 193],
                        pt_pair[p][:, 512:1024],
                        start=(kt == 0), stop=(kt == KT - 1),
                    )

            # ---- attention: scores/exp/mask at kt, PV two kt behind (the
            # lag keeps PE fed across DVE epilogue bursts and injections);
            # the previous qt's epilogue is emitted inside this qt's kt=1
            # so its DVE copies land behind kt0's multiplies ----
            def make_epilogue(qt, po):
                qsl = slice(qt * 512, (qt + 1) * 512)

                def emit(last=False):
                    # ou layout per p: [:, 0, :] rows 0:65 = A block + sumA
                    # row 64; [:, 1, :] rows 32:128 = sumB row 32 + vhB rows
                    # 64:128 (disjoint free columns, so no partition clash)
                    dma = nc.sync
                    ou = singles.tile([128, 2, 2, 512], BF16, tag="ou",
                                      name="ou", bufs=2)
                    for p in range(2):
                        nc.vector.tensor_copy(
                            out=ou[64:65, p, 0, :], in_=po[p][64:65, 0:512])
                        nc.vector.tensor_copy(
                            out=ou[32:33, p, 1, :], in_=po[p][32:33, 512:1024])
                    recin = singles.tile([128, 16], BF16, tag="recin",
                                         name="recin", bufs=2)
                    for h in range(HPC):
                        p, is_b = h // 2, h % 2
                        row = 32 if is_b else 64
                        dma.dma_start(
                            out=recin[:, h * 4:(h + 1) * 4],
                            in_=ou[row:row + 1, p, is_b, :])
                    for p in range(2):
                        nc.vector.tensor_copy(
                            out=ou[0:64, p, 0, :], in_=po[p][0:64, 0:512])
                        nc.vector.tensor_copy(
                            out=ou[64:128, p, 1, :], in_=po[p][64:128, 512:1024])
                    recout = singles.tile([128, 16], BF16, tag="recout",
                                          name="recout", bufs=2)
                    with nc.allow_low_precision(
                            reason="1/rowsum broadcast factor; bf16 is plenty"):
                        nc.vector.reciprocal(out=recout, in_=recin)
                    for h in range(HPC):
                        dma.dma_start(
                            out=rscr_d[h:h + 1, qsl],
                            in_=recout[:, h * 4:(h + 1) * 4])
                    rbc = singles.tile([128, 2, 512], BF16, tag="rbc",
                                       name="rbc", bufs=2)
                    for p in range(2):
                        for ab in range(2):
                            srow = rscr_d[2 * p + ab:2 * p + ab + 1, qsl]
                            src_bc = bass.AP(
                                tensor=srow.tensor,
                                offset=srow.offset,
                                ap=[[0, 64], list(srow.ap[-1])],
                            )
                            dma.dma_start(
                                out=rbc[ab * 64:(ab + 1) * 64, p, :], in_=src_bc)
                    tt_eng = nc.vector if last else nc.gpsimd
                    for p in range(2):
                        tt_eng.tensor_tensor(
                            out=OT[p][0:64, qsl],
                            in0=ou[0:64, p, 0, :],
                            in1=rbc[0:64, p, :],
                            op=mybir.AluOpType.mult)
                        tt_eng.tensor_tensor(
                            out=OT[p][64:128, qsl],
                            in0=ou[64:128, p, 1, :],
                            in1=rbc[64:128, p, :],
                            op=mybir.AluOpType.mult)

                return emit

            pending_epilogue = None
            for qt in range(QT):
                qsl = slice(qt * 512, (qt + 1) * 512)
                po = [pvp.tile([128, 1024], F32, tag="po", name="po")
                      for _ in range(2)]
                pt_hist = []
                for kt in range(KT + 2):
                    if kt == 1 and pending_epilogue is not None:
                        pending_epilogue()
                        pending_epilogue = None
                    if kt < KT:
                        # just-in-time DMA staging + deferred projections,
                        # split into half-chunks to smooth PE disruption
                        if qt == 0:
                            if kt == 0:
                                load_x_chunk(xk_d, xk_sb, 2, [nc.gpsimd])
                            elif kt == 2:
                                load_x_chunk(xk_d, xk_sb, 3, [nc.gpsimd])
                            if kt in (2, 4):
                                proj_qk(wk_sb, bk_sb, khT, kt // 3, 1, xk_sb)
                            elif kt in (5, 7):
                                proj_qk(wk_sb, bk_sb, khT, kt // 6, 2, xk_sb)
                            elif kt in (9, 11):
                                proj_qk(wk_sb, bk_sb, khT, kt // 10, 3, xk_sb)
                        if qt < QT - 1:
                            if kt == 8:
                                load_x_chunk(xq_d, xq_sb, qt + 1, [nc.gpsimd])
                            elif kt in (10, 13):
                                proj_qk(wq_sb, bq_sb, qhT, kt // 11, qt + 1, xq_sb)

                        mt8 = singles.tile([128, 512], F8, tag="mask8",
                                           name="mask8", bufs=8)
                        mask_eng = nc.scalar if (qt == 0 and kt < 8) else nc.sync
                        mask_eng.dma_start(out=mt8, in_=mk_d[kt, qt])
                        mt = singles.tile([128, 512], BF16, tag="mask",
                                          name="mask", bufs=4)
                        nc.gpsimd.tensor_copy(out=mt, in_=mt8)
                        m_ap = mt[:, :]
                        mbc = bass.AP(
                            tensor=m_ap.tensor,
                            offset=m_ap.offset,
                            ap=[list(m_ap.ap[0]), [0, 2], list(m_ap.ap[1])],
                        )
                        cur_pt = []
                        for p in range(2):
                            ps = scp.tile([128, 1024], F32, tag="sc", name="ps")
                            for ab in range(2):
                                nc.tensor.matmul(
                                    ps[:, ab * 512:(ab + 1) * 512],
                                    khT[p][kt // 4][ab * 64:(ab + 1) * 64,
                                                    (kt % 4) * 128:(kt % 4 + 1) * 128],
                                    qhT[p][qt][ab * 64:(ab + 1) * 64, :],
                                    start=True,
                                    stop=True,
                                )
                            pt = singles.tile([128, 1024], BF16, tag="pt",
                                              name="pt", bufs=8)
                            nc.scalar.activation(
                                out=pt, in_=ps,
                                func=mybir.ActivationFunctionType.Exp,
                                scale=float(SCALE / (W8SCALE * W8SCALE)),
                            )
                            nc.vector.tensor_tensor(
                                out=pt, in0=pt, in1=mbc,
                                op=mybir.AluOpType.mult,
                            )
                            cur_pt.append(pt)
                        pt_hist.append(cur_pt)
                    if kt >= 2:
                        pv_step(po, pt_hist[kt - 2], kt - 2)
                pending_epilogue = make_epilogue(qt, po)
            pending_epilogue(last=True)

            # ---- output projection (attention psum rings are free now):
            # y[q, :] = sum_p OT[p][:, qchunk].T @ wo_sb[p]
            for i in range(16):
                qt, qc = i // 4, i % 4
                pool = scp if i % 2 == 0 else pvp
                tag = "sc" if i % 2 == 0 else "po"
                ps = pool.tile([128, 1024], F32, tag=tag, name="psy")
                for p in range(2):
                    for n in range(2):
                        nc.tensor.matmul(
                            ps[:, n * 512:(n + 1) * 512],
                            OT[p][:, qt * 512 + qc * 128:
                                  qt * 512 + (qc + 1) * 128],
                            wo_sb[p][:, n * 512:(n + 1) * 512],
                            start=(p == 0),
                            stop=(p == 1),
                        )
                yt = singles.tile([128, 1024], BF16, tag="yt",
                                  name="yt", bufs=4)
                if i % 2 == 0:
                    nc.vector.tensor_copy(out=yt, in_=ps)
                else:
                    nc.scalar.copy(out=yt, in_=ps)
                nc.sync.dma_start(out=yt_d[qt, qc], in_=yt)
    nc.compile()
    return nc


_NC_CACHE = None


def get_nc():
    global _NC_CACHE
    if _NC_CACHE is None:
        _NC_CACHE = build_nc()
    return _NC_CACHE


def _tile_x(xT):
    # [1024, 2048] -> [QT, CT, 128, 512]
    return np.ascontiguousarray(
        xT.reshape(CT, 128, QT, 512).transpose(2, 0, 1, 3))


def prep_in_maps(q, k, v, mask, Wq, bq, Wk, bk, Wv, bv, Wo, bo):
    q = np.asarray(q, np.float32)
    k = np.asarray(k, np.float32)
    v = np.asarray(v, np.float32)
    mask = np.asarray(mask)
    WqT = np.asarray(Wq, np.float32).T
    WkT = np.asarray(Wk, np.float32).T
    WvT = np.asarray(Wv, np.float32).T
    WoT = np.asarray(Wo, np.float32).T
    bq = np.asarray(bq, np.float32)
    bk = np.asarray(bk, np.float32)
    bv = np.asarray(bv, np.float32)

    xT = {}
    keepT = {}
    for b in range(B):
        xT[b] = (
            _tile_x(np.ascontiguousarray(q[b].T).astype(NP_F8)),
            _tile_x(np.ascontiguousarray(k[b].T).astype(NP_F8)),
            _tile_x(np.ascontiguousarray(v[b].T).astype(NP_BF16)),
        )
        mt = np.ascontiguousarray((~mask[b, 0]).T.astype(np.float32)).astype(NP_F8)
        keepT[b] = np.ascontiguousarray(
            mt.reshape(KT, 128, QT, 512).transpose(0, 2, 1, 3))

    in_maps = []
    for c in range(N_CORES):
        b = c // 4
        ho = c % 4
        dsl = slice(ho * 256, ho * 256 + 256)
        xq, xk, xv = xT[b]
        in_maps.append({
            "xq": xq,
            "xk": xk,
            "xv": xv,
            "wq": np.ascontiguousarray(WqT[:, dsl] * W8SCALE).astype(NP_F8).reshape(CT, 128, 256),
            "wk": np.ascontiguousarray(WkT[:, dsl] * W8SCALE).astype(NP_F8).reshape(CT, 128, 256),
            "wv": np.ascontiguousarray(WvT[:, dsl]).astype(NP_BF16).reshape(CT, 128, 256),
            "wo": np.ascontiguousarray(WoT[dsl, :]).astype(NP_BF16).reshape(2, 128, 1024),
            "bq2": np.ascontiguousarray(bq[dsl] * W8SCALE).reshape(2, 128, 1).astype(np.float32),
            "bk2": np.ascontiguousarray(bk[dsl] * W8SCALE).reshape(2, 128, 1).astype(np.float32),
            "bvb": np.ascontiguousarray(
                np.broadcast_to(bv[dsl], (128, 256))).astype(NP_BF16),
            "mk": keepT[b],
        })
    return in_maps


def gather_output(results, bo):
    bo = np.asarray(bo, np.float32)
    y = np.zeros((B, S, DIM), np.float32)
    for c in range(N_CORES):
        yt = np.asarray(results[c]["yt"], np.float32)  # [QT, 4, 128, 1024]
        y[c // 4] += yt.reshape(S, DIM)
    y += bo[None, None, :]
    return y


def kernel(**inputs):
    nc = get_nc()
    in_maps = prep_in_maps(**{k_: inputs[k_] for k_ in (
        "q", "k", "v", "mask", "Wq", "bq", "Wk", "bk", "Wv", "bv", "Wo", "bo")})
    res = bass_utils.run_bass_kernel_spmd(nc, in_maps, list(range(N_CORES)))
    return gather_output(res.results, inputs["bo"])


# revision 30
# speedup vs baseline: 1.2245x; 1.2245x over previous
"""MultiHeadAttention Trainium2 Bass kernel (8-core SPMD), v4.

Problem: B=2, S=2048, DIM=1024, H=16 heads (dh=64), fp32 reference.
Sharding: core c handles batch b = c//4 and 4 heads ho = 4*(c%4)..+4
(data-parallel over batch x tensor-parallel over heads). Each core:
  qhT/khT = W{q,k}.T-slice @ x.T + b   -> [256, 2048] bf16 (head-dim major)
  vh_aug  = x @ Wv.T-slice + bv (+ones col per head) per k-tile
  scores^T = kh @ qh.T (per head, K=64 row-packed pairs)
  P^T = keepmask * exp(SCALE * scores^T)                (ACT + DVE)
  out^T[65|128, q] = [vh | ones].T @ P^T                (PV + row-sums fused)
  normalize by 1/sums, y[q, :] = sum_p OT[p]-chunk.T @ Wo.T-slice

The attention inner loop is ACT-bound (exp ~2.03us/kt); everything else
is arranged to hide under it:
  - input DMAs are issue-rate-bound (~0.65us/dma_start per sequencer),
    so the head preloads are spread across the scalar/vector/gpsimd
    sequencers; sync keeps the in-loop mask stream.
  - PV matmuls are software-pipelined one kt behind the score matmuls so
    the PE FIFO never stalls waiting for exp+mask of the current kt.
  - only V proj, K chunk 0 and Q chunk 0 gate attention start; K chunks
    1-3 / Q chunks 1-3 are injected into the attention loop.
  - per-qt epilogue: po is drained to SBUF by GPSIMD (off the DVE queue)
    so the psum ring frees in ~2us; the bf16 reciprocal-broadcast
    round-trip and OT normalize run hidden under the next qt.
  - output projection at the end uses OT 128-col chunks as lhsT -> y is
    produced q-major (host gather is a plain reshape+add), alternating
    between both psum rings and both cast engines.
"""

import os
import sys

sys.path.insert(0, "/opt/trn_rl_repo")
os.environ.setdefault("MYCRO_LOCAL_CACHE", "1")

import numpy as np

import concourse.bass as bass
import concourse.bacc as bacc
import concourse.tile as tile
from concourse import mybir
from concourse import bass_utils

F32 = mybir.dt.float32
BF16 = mybir.dt.bfloat16
F8 = mybir.dt.float8e4
NP_BF16 = mybir.dt.np(BF16)
NP_F8 = mybir.dt.np(F8)
W8SCALE = 16.0  # wq/wk prescale so fp8-e4m3 stays in normal range

B, S, DIM = 2, 2048, 1024
H = 16
DH = 64
SCALE = 1.0 / (DIM ** 0.5)
N_CORES = 8
HPC = 4          # heads per core
QT = S // 512    # 4 q-chunks of 512
KT = S // 128    # 16 k-tiles of 128
CT = DIM // 128  # 8 contraction tiles for projections

# vh_aug per-kt layout: per pair p (2 local pairs):
#   A block: [vh_A(64) | ones(1)]                 at cols p*193 + [0, 65)
#   B block: [zeros(32) | ones(1) | zeros(31) | vh_B(64)] at cols p*193 + [65, 193)
#   (B ones at col +97 so B sums land on psum partition 32 -- DVE start
#   partitions must be in {0, 32, 64, 96})
VHA_W = 386


def build_nc():
    # Bacc (not plain Bass): its compile() pipeline splits multi-semaphore
    # waits into event-semaphore chains -- walrus codegen allows only ONE
    # sync wait per compute instruction on TRN2.
    nc = bacc.Bacc("TRN2", target_bir_lowering=False)

    xq_d = nc.declare_dram_parameter("xq", [QT, CT, 128, 512], F8, isOutput=False)
    xk_d = nc.declare_dram_parameter("xk", [QT, CT, 128, 512], F8, isOutput=False)
    xv_d = nc.declare_dram_parameter("xv", [QT, CT, 128, 512], BF16, isOutput=False)
    wq_d = nc.declare_dram_parameter("wq", [CT, 128, 256], F8, isOutput=False)
    wk_d = nc.declare_dram_parameter("wk", [CT, 128, 256], F8, isOutput=False)
    wv_d = nc.declare_dram_parameter("wv", [CT, 128, 256], BF16, isOutput=False)
    wo_d = nc.declare_dram_parameter("wo", [2, 128, 1024], BF16, isOutput=False)
    bq_d = nc.declare_dram_parameter("bq2", [2, 128, 1], F32, isOutput=False)
    bk_d = nc.declare_dram_parameter("bk2", [2, 128, 1], F32, isOutput=False)
    bvb_d = nc.declare_dram_parameter("bvb", [128, 256], BF16, isOutput=False)
    mk_d = nc.declare_dram_parameter("mk", [KT, QT, 128, 512], BF16, isOutput=False)
    # y chunks, q-major: [qt, qc, 128 q, 1024 dims]
    yt_d = nc.declare_dram_parameter("yt", [QT, 4, 128, 1024], BF16, isOutput=True)
    rscr_d = nc.dram_tensor("rscr", [HPC, S], BF16)

    with tile.TileContext(nc) as tc:
        with tc.tile_pool(name="persist", bufs=1) as singles, \
             tc.tile_pool(name="scp", bufs=2, space="PSUM") as scp, \
             tc.tile_pool(name="pvp", bufs=2, space="PSUM") as pvp:
            def load_rows(dram, n_tiles, width, tag, eng, dt=BF16):
                tiles = []
                for c in range(n_tiles):
                    t = singles.tile([128, width], dt, tag=f"{tag}{c}", name=f"{tag}{c}")
                    eng.dma_start(out=t, in_=dram[c])
                    tiles.append(t)
                return tiles

            def alloc_x(tag, dt=BF16):
                return [[singles.tile([128, 512], dt,
                                      tag=f"{tag}{c}_{n}", name=f"{tag}{c}_{n}")
                         for n in range(QT)] for c in range(CT)]

            def load_x_chunk(dram, tiles, n, engines):
                # all 8 c-tiles of one 512-column chunk, split across engines
                for c in range(CT):
                    engines[c % len(engines)].dma_start(
                        out=tiles[c][n], in_=dram[n, c])

            xv_sb = alloc_x("xv")
            xk_sb = alloc_x("xk", dt=F8)
            xq_sb = alloc_x("xq", dt=F8)
            # DMA issue is ~0.65us/dma_start per sequencer, so spread the
            # head across sync+scalar+gpsimd. Small-but-gating tiles
            # (biases) go absolutely first; then xv (V proj), then the
            # K0/Q0 inputs; trailing weights last.
            bvb_sb = singles.tile([128, 256], BF16, tag="bvb")
            nc.sync.dma_start(out=bvb_sb, in_=bvb_d[:, :])
            bq_sb, bk_sb = [], []
            for m in range(2):
                tq = singles.tile([128, 1], F32, tag=f"bq{m}", name=f"bq{m}")
                nc.sync.dma_start(out=tq, in_=bq_d[m])
                bq_sb.append(tq)
                tk = singles.tile([128, 1], F32, tag=f"bk{m}", name=f"bk{m}")
                nc.sync.dma_start(out=tk, in_=bk_d[m])
                bk_sb.append(tk)
            wv_sb = load_rows(wv_d, CT, 256, "wv", nc.scalar)
            for n in range(QT):
                load_x_chunk(xv_d, xv_sb, n, [nc.scalar, nc.sync])
            wk_sb = load_rows(wk_d, CT, 256, "wk", nc.gpsimd, dt=F8)
            load_x_chunk(xk_d, xk_sb, 0, [nc.gpsimd])
            wq_sb = load_rows(wq_d, CT, 256, "wq", nc.gpsimd, dt=F8)
            load_x_chunk(xq_d, xq_sb, 0, [nc.gpsimd])
            load_x_chunk(xk_d, xk_sb, 1, [nc.gpsimd])
            wo_sb = load_rows(wo_d, 2, DIM, "wo", nc.sync)

            # ---- persistent intermediates ----
            qhT = [[singles.tile([128, 512], BF16, tag=f"qhT{m}_{n}",
                                 name=f"qhT{m}_{n}") for n in range(QT)]
                   for m in range(2)]
            khT = [[singles.tile([128, 512], BF16, tag=f"khT{m}_{n}",
                                 name=f"khT{m}_{n}") for n in range(QT)]
                   for m in range(2)]
            OT = [singles.tile([128, S], BF16, tag=f"OT{m}", name=f"OT{m}") for m in range(2)]
            vha = [singles.tile([128, VHA_W], BF16, tag=f"vha{kt}",
                                name=f"vha{kt}") for kt in range(KT)]

            # memsets on DVE (gpsimd's queue is busy issuing DMAs)
            warm = singles.tile([128, 1], BF16, tag="warm")
            nc.vector.memset(warm[:, :], 0.0)
            # preload the exp table set off the critical path
            dummy = singles.tile([128, 1], BF16, tag="dummy")
            nc.scalar.activation(
                out=dummy, in_=warm[:, 0:1],
                func=mybir.ActivationFunctionType.Exp, scale=1.0)
            for kt in range(KT):
                for p in range(2):
                    base = p * 193
                    nc.vector.memset(vha[kt][:, base + 64:base + 65], 1.0)
                    nc.vector.memset(vha[kt][:, base + 97:base + 98], 1.0)
                    nc.vector.memset(vha[kt][:, base + 65:base + 97], 0.0)
                    nc.vector.memset(vha[kt][:, base + 98:base + 129], 0.0)

            # ---- V projection (vh_aug tiles) ----
            def v_proj(kt, pool=None, tag="po"):
                pool = pvp if pool is None else pool
                ps = pool.tile([128, 256], F32, tag=tag, name="psv")
                for c in range(CT):
                    nc.tensor.matmul(
                        ps,
                        xv_sb[c][kt // 4][:, (kt % 4) * 128:(kt % 4 + 1) * 128],
                        wv_sb[c],
                        start=(c == 0),
                        stop=(c == CT - 1),
                    )
                for h in range(HPC):
                    p, is_b = h // 2, h % 2
                    col = p * 193 + (129 if is_b else 0)
                    nc.vector.tensor_tensor(
                        out=vha[kt][:, col:col + 64],
                        in0=ps[:, h * 64:(h + 1) * 64],
                        in1=bvb_sb[:, h * 64:(h + 1) * 64],
                        op=mybir.AluOpType.add,
                    )

            def proj_qk(w_sb, b_sb, dst, m, n, x_sb):
                ps = scp.tile([128, 512], F32, tag="sc", name="psqk")
                for c in range(CT):
                    nc.tensor.matmul(
                        ps,
                        w_sb[c][:, m * 128:(m + 1) * 128],
                        x_sb[c][n],
                        start=(c == 0),
                        stop=(c == CT - 1),
                    )
                bb = b_sb[m][:, 0:1]
                bb_bc = bass.AP(
                    tensor=bb.tensor, offset=bb.offset,
                    ap=[list(bb.ap[0]), [0, 512]])
                nc.vector.tensor_tensor(
                    out=dst[m][n],
                    in0=ps,
                    in1=bb_bc,
                    op=mybir.AluOpType.add,
                )

            # ---- head: V interleaved with K chunk 0 + Q chunk 0; the rest
            # of K/Q is injected into the attention loop ----
            for kt in range(8):
                v_proj(kt)
            proj_qk(wk_sb, bk_sb, khT, 0, 0, xk_sb)
            for kt in range(8, 12):
                v_proj(kt)
            proj_qk(wk_sb, bk_sb, khT, 1, 0, xk_sb)
            for kt in range(12, KT):
                v_proj(kt)
            for m in range(2):
                proj_qk(wq_sb, bq_sb, qhT, m, 0, xq_sb)

            def pv_step(po, pt_pair, kt):
                for p in range(2):
                    base = p * 193
                    nc.tensor.matmul(
                        po[p][:, 0:512],
                        vha[kt][:, base:base + 128],
                        pt_pair[p][:, 0:512],
                        start=(kt == 0), stop=(kt == KT - 1),
                    )
                    nc.tensor.matmul(
                        po[p][:, 512:1024],
                        vha[kt][:, base + 65:base + 193],
                        pt_pair[p][:, 512:1024],
                        start=(kt == 0), stop=(kt == KT - 1),
                    )

            # ---- attention: scores/exp/mask at kt, PV two kt behind (the
            # lag keeps PE fed across DVE epilogue bursts and injections);
            # the previous qt's epilogue is emitted inside this qt's kt=1
            # so its DVE copies land behind kt0's multiplies ----
            def make_epilogue(qt, po):
                qsl = slice(qt * 512, (qt + 1) * 512)

                def emit(last=False):
                    # ou layout per p: [:, 0, :] rows 0:65 = A block + sumA
                    # row 64; [:, 1, :] rows 32:128 = sumB row 32 + vhB rows
                    # 64:128 (disjoint free columns, so no partition clash)
                    dma = nc.sync
                    ou = singles.tile([128, 2, 2, 512], BF16, tag="ou",
                                      name="ou", bufs=2)
                    for p in range(2):
                        nc.vector.tensor_copy(
                            out=ou[64:65, p, 0, :], in_=po[p][64:65, 0:512])
                        nc.vector.tensor_copy(
                            out=ou[32:33, p, 1, :], in_=po[p][32:33, 512:1024])
                    recin = singles.tile([128, 16], BF16, tag="recin",
                                         name="recin", bufs=2)
                    for h in range(HPC):
                        p, is_b = h // 2, h % 2
                        row = 32 if is_b else 64
                        dma.dma_start(
                            out=recin[:, h * 4:(h + 1) * 4],
                            in_=ou[row:row + 1, p, is_b, :])
                    for p in range(2):
                        nc.vector.tensor_copy(
                            out=ou[0:64, p, 0, :], in_=po[p][0:64, 0:512])
                        nc.vector.tensor_copy(
                            out=ou[64:128, p, 1, :], in_=po[p][64:128, 512:1024])
                    recout = singles.tile([128, 16], BF16, tag="recout",
                                          name="recout", bufs=2)
                    with nc.allow_low_precision(
                            reason="1/rowsum broadcast factor; bf16 is plenty"):
                        nc.vector.reciprocal(out=recout, in_=recin)
                    for h in range(HPC):
                        dma.dma_start(
                            out=rscr_d[h:h + 1, qsl],
                            in_=recout[:, h * 4:(h + 1) * 4])
                    rbc = singles.tile([128, 2, 512], BF16, tag="rbc",
                                       name="rbc", bufs=2)
                    for p in range(2):
                        for ab in range(2):
                            srow = rscr_d[2 * p + ab:2 * p + ab + 1, qsl]
                            src_bc = bass.AP(
                                tensor=srow.tensor,
                                offset=srow.offset,
                                ap=[[0, 64], list(srow.ap[-1])],
                            )
                            dma.dma_start(
                                out=rbc[ab * 64:(ab + 1) * 64, p, :], in_=src_bc)
                    tt_eng = nc.vector if last else nc.gpsimd
                    for p in range(2):
                        tt_eng.tensor_tensor(
                            out=OT[p][0:64, qsl],
                            in0=ou[0:64, p, 0, :],
                            in1=rbc[0:64, p, :],
                            op=mybir.AluOpType.mult)
                        tt_eng.tensor_tensor(
                            out=OT[p][64:128, qsl],
                            in0=ou[64:128, p, 1, :],
                            in1=rbc[64:128, p, :],
                            op=mybir.AluOpType.mult)

                return emit

            pending_epilogue = None
            for qt in range(QT):
                qsl = slice(qt * 512, (qt + 1) * 512)
                po = [pvp.tile([128, 1024], F32, tag="po", name="po")
                      for _ in range(2)]
                pt_hist = []
                for kt in range(KT + 2):
                    if kt == 1 and pending_epilogue is not None:
                        pending_epilogue()
                        pending_epilogue = None
                    if kt < KT:
                        # just-in-time DMA staging + deferred projections,
                        # split into half-chunks to smooth PE disruption
                        if qt == 0:
                            if kt == 0:
                                load_x_chunk(xk_d, xk_sb, 2, [nc.gpsimd])
                            elif kt == 2:
                                load_x_chunk(xk_d, xk_sb, 3, [nc.gpsimd])
                            if kt in (2, 4):
                                proj_qk(wk_sb, bk_sb, khT, kt // 3, 1, xk_sb)
                            elif kt in (5, 7):
                                proj_qk(wk_sb, bk_sb, khT, kt // 6, 2, xk_sb)
                            elif kt in (9, 11):
                                proj_qk(wk_sb, bk_sb, khT, kt // 10, 3, xk_sb)
                        if qt < QT - 1:
                            if kt == 8:
                                load_x_chunk(xq_d, xq_sb, qt + 1, [nc.gpsimd])
                            elif kt in (10, 13):
                                proj_qk(wq_sb, bq_sb, qhT, kt // 11, qt + 1, xq_sb)

                        mt = singles.tile([128, 512], BF16, tag="mask",
                                          name="mask", bufs=8)
                        mask_eng = nc.scalar if (qt == 0 and kt < 8) else nc.sync
                        mask_eng.dma_start(out=mt, in_=mk_d[kt, qt])
                        m_ap = mt[:, :]
                        mbc = bass.AP(
                            tensor=m_ap.tensor,
                            offset=m_ap.offset,
                            ap=[list(m_ap.ap[0]), [0, 2], list(m_ap.ap[1])],
                        )
                        cur_pt = []
                        for p in range(2):
                            ps = scp.tile([128, 1024], F32, tag="sc", name="ps")
                            for ab in range(2):
                                nc.tensor.matmul(
                                    ps[:, ab * 512:(ab + 1) * 512],
                                    khT[p][kt // 4][ab * 64:(ab + 1) * 64,
                                                    (kt % 4) * 128:(kt % 4 + 1) * 128],
                                    qhT[p][qt][ab * 64:(ab + 1) * 64, :],
                                    start=True,
                                    stop=True,
                                )
                            pt = singles.tile([128, 1024], BF16, tag="pt",
                                              name="pt", bufs=8)
                            nc.scalar.activation(
                                out=pt, in_=ps,
                                func=mybir.ActivationFunctionType.Exp,
                                scale=float(SCALE / (W8SCALE * W8SCALE)),
                            )
                            nc.vector.tensor_tensor(
                                out=pt, in0=pt, in1=mbc,
                                op=mybir.AluOpType.mult,
                            )
                            cur_pt.append(pt)
                        pt_hist.append(cur_pt)
                    if kt >= 2:
                        pv_step(po, pt_hist[kt - 2], kt - 2)
                pending_epilogue = make_epilogue(qt, po)
            pending_epilogue(last=True)

            # ---- output projection (attention psum rings are free now):
            # y[q, :] = sum_p OT[p][:, qchunk].T @ wo_sb[p]
            for i in range(16):
                qt, qc = i // 4, i % 4
                pool = scp if i % 2 == 0 else pvp
                tag = "sc" if i % 2 == 0 else "po"
                ps = pool.tile([128, 1024], F32, tag=tag, name="psy")
                for p in range(2):
                    for n in range(2):
                        nc.tensor.matmul(
                            ps[:, n * 512:(n + 1) * 512],
                            OT[p][:, qt * 512 + qc * 128:
                                  qt * 512 + (qc + 1) * 128],
                            wo_sb[p][:, n * 512:(n + 1) * 512],
                            start=(p == 0),
                            stop=(p == 1),
                        )
                yt = singles.tile([128, 1024], BF16, tag="yt",
                                  name="yt", bufs=4)
                if i % 2 == 0:
                    nc.vector.tensor_copy(out=yt, in_=ps)
                else:
                    nc.scalar.copy(out=yt, in_=ps)
                nc.sync.dma_start(out=yt_d[qt, qc], in_=yt)
    nc.compile()
    return nc


_NC_CACHE = None


def get_nc():
    global _NC_CACHE
    if _NC_CACHE is None:
        _NC_CACHE = build_nc()
    return _NC_CACHE


def _tile_x(xT):
    # [1024, 2048] -> [QT, CT, 128, 512]
    return np.ascontiguousarray(
        xT.reshape(CT, 128, QT, 512).transpose(2, 0, 1, 3))


def prep_in_maps(q, k, v, mask, Wq, bq, Wk, bk, Wv, bv, Wo, bo):
    q = np.asarray(q, np.float32)
    k = np.asarray(k, np.float32)
    v = np.asarray(v, np.float32)
    mask = np.asarray(mask)
    WqT = np.asarray(Wq, np.float32).T
    WkT = np.asarray(Wk, np.float32).T
    WvT = np.asarray(Wv, np.float32).T
    WoT = np.asarray(Wo, np.float32).T
    bq = np.asarray(bq, np.float32)
    bk = np.asarray(bk, np.float32)
    bv = np.asarray(bv, np.float32)

    xT = {}
    keepT = {}
    for b in range(B):
        xT[b] = (
            _tile_x(np.ascontiguousarray(q[b].T).astype(NP_F8)),
            _tile_x(np.ascontiguousarray(k[b].T).astype(NP_F8)),
            _tile_x(np.ascontiguousarray(v[b].T).astype(NP_BF16)),
        )
        mt = np.ascontiguousarray((~mask[b, 0]).T.astype(np.float32)).astype(NP_BF16)
        keepT[b] = np.ascontiguousarray(
            mt.reshape(KT, 128, QT, 512).transpose(0, 2, 1, 3))

    in_maps = []
    for c in range(N_CORES):
        b = c // 4
        ho = c % 4
        dsl = slice(ho * 256, ho * 256 + 256)
        xq, xk, xv = xT[b]
        in_maps.append({
            "xq": xq,
            "xk": xk,
            "xv": xv,
            "wq": np.ascontiguousarray(WqT[:, dsl] * W8SCALE).astype(NP_F8).reshape(CT, 128, 256),
            "wk": np.ascontiguousarray(WkT[:, dsl] * W8SCALE).astype(NP_F8).reshape(CT, 128, 256),
            "wv": np.ascontiguousarray(WvT[:, dsl]).astype(NP_BF16).reshape(CT, 128, 256),
            "wo": np.ascontiguousarray(WoT[dsl, :]).astype(NP_BF16).reshape(2, 128, 1024),
            "bq2": np.ascontiguousarray(bq[dsl] * W8SCALE).reshape(2, 128, 1).astype(np.float32),
            "bk2": np.ascontiguousarray(bk[dsl] * W8SCALE).reshape(2, 128, 1).astype(np.float32),
            "bvb": np.ascontiguousarray(
                np.broadcast_to(bv[dsl], (128, 256))).astype(NP_BF16),
            "mk": keepT[b],
        })
    return in_maps


def gather_output(results, bo):
    bo = np.asarray(bo, np.float32)
    y = np.zeros((B, S, DIM), np.float32)
    for c in range(N_CORES):
        yt = np.asarray(results[c]["yt"], np.float32)  # [QT, 4, 128, 1024]
        y[c // 4] += yt.reshape(S, DIM)
    y += bo[None, None, :]
    return y


def kernel(**inputs):
    nc = get_nc()
    in_maps = prep_in_maps(**{k_: inputs[k_] for k_ in (
        "q", "k", "v", "mask", "Wq", "bq", "Wk", "bk", "Wv", "bv", "Wo", "bo")})
    res = bass_utils.run_bass_kernel_spmd(nc, in_maps, list(range(N_CORES)))
    return gather_output(res.results, inputs["bo"])
